# revision 10
# baseline (speedup 1.0000x reference)
"""Biaffine edge attention on 8 Trainium2 NeuronCores (fp16, PE-roofline schedule).

Math (per batch b):
    out[i,o] = head[i,:] @ U @ dep[o,:] + head[i,:]@wh + dep[o,:]@wd + b
with head/dep [S=2048, D=256], U [D,D], edge_W = [wh | wd] (each [D]).

Sharding: pure data-parallel over batch B=8 -> one batch per core,
U / edge_W / edge_b replicated. No collectives.

Host prep (layout only + the tiny rank-1 bias):
    headT/depT: inputs pre-transposed to [D, S] fp16 -- the PE needs the
        contraction dim on partitions for both operands, and host-side
        layout beats 64 PE transposes + 16 PSUM-collect copies on device.
    hs2[p, j] = head[j*128+p, :] @ wh + b   (per-row bias, [128, 16] f32)
    u2 = [U[:128, :] | U[128:, :]]          ([128, 512] fp16)
    wd2[p, eb] = wd[eb*128+p]               ([128, 2] f32)

Per-core kernel (fp16 matmuls, f32 PSUM, fp16 stores upcast on host):
    ATf[e,i] = sum_d U[d,e] headT[d,i] + wd[e]    (ds[o] rides the
               e-contraction of the out matmul for free)
    out[i,o] = sum_e ATf[e,i] depT[e,o] + hs2[i]  (bias fused in the
               PSUM->SBUF eviction on ACT/DVE)

Schedule notes:
  - headT loads on the ACT HWDGE ring, depT on the SP ring (parallel
    descriptor streams; SDMA round-robins), consts on the Pool ring.
  - ~8 junk matmuls on a memset tile warm the PE HAM clock gate
    (1.2 -> 2.4 GHz needs ~3.4us of sustained busy) during the loads.
  - single 8-buf PSUM pool of [128,512] f32 (= all 8 banks); out
    row-blocks use 4 banks each, eb-outer so only 2 LDWEIGHTS/row.
  - epilogue alternates DVE/ACT per 512-chunk; row bias (hs2) and
    ATf bias (wd2) ride the eviction for free.
  - stores: one [128,2048] fp16 DMA per row-block on the SP ring.
"""

import numpy as np

import concourse.bass as bass
import concourse.tile as tile
from concourse import bacc, mybir
from concourse.bass_utils import run_bass_kernel_spmd

B, S, D = 8, 2048, 256
P = 128          # partitions
OC = 512         # matmul moving free-dim chunk (one PSUM bank of fp32)
NI = S // P      # 16 output row blocks
NC = S // OC     # 4 output column chunks
ND = D // P      # 2 contraction chunks
NWARM = 7        # PE warm-up filler matmuls (cover load latency, warm HAM)
F32 = mybir.dt.float32
F16 = mybir.dt.float16

Ident = mybir.ActivationFunctionType.Identity


def build_nc(reps=1):
    """reps>1 wraps the body in a HW For_i loop -- used only for timing."""
    nc = bacc.Bacc("TRN2", target_bir_lowering=False, debug=False, num_devices=B)

    headT_d = nc.dram_tensor("headT", [D, S], F16, kind="ExternalInput")
    depT_d = nc.dram_tensor("depT", [D, S], F16, kind="ExternalInput")
    u2_d = nc.dram_tensor("u2", [P, ND * D], F16, kind="ExternalInput")
    wd2_d = nc.dram_tensor("wd2", [P, ND], F32, kind="ExternalInput")
    hs2_d = nc.dram_tensor("hs2", [P, NI], F32, kind="ExternalInput")
    out_d = nc.dram_tensor("out", [S, S], F16, kind="ExternalOutput")

    with tile.TileContext(nc) as tc:
        with (
            tc.tile_pool(name="const", bufs=1) as cpool,
            tc.tile_pool(name="persist", bufs=1) as ppool,
            tc.tile_pool(name="outbuf", bufs=3) as outbuf,
            tc.tile_pool(name="ps", bufs=7, space=bass.MemorySpace.PSUM) as ps,
            tc.tile_pool(name="psw", bufs=1, space=bass.MemorySpace.PSUM) as psw,
        ):
            def body():
                # ---- ALL input loads on the ACT HWDGE ring, in priority
                #      order (u2, head, dep).  One ring = FIFO-serialized
                #      transfers, each at full SDMA rate -- concurrent rings
                #      would packet-interleave and delay the head loads the
                #      ATf phase is waiting on.  dep is only needed when the
                #      first out row-block starts, so it rides last.  The SP
                #      ring is left free for the output stores. ----
                u2 = cpool.tile([P, ND * D], F16, name="u2", tag="u2")
                nc.scalar.dma_start(u2[:], u2_d[:])
                wd2 = cpool.tile([P, ND], F32, name="wd2", tag="wd2")
                nc.gpsimd.dma_start(wd2[:], wd2_d[:])
                hs2 = cpool.tile([P, NI], F32, name="hs2", tag="hs2")
                nc.gpsimd.dma_start(hs2[:], hs2_d[:])

                headT = [ppool.tile([P, S], F16, name=f"headT{dc}",
                                    tag=f"headT{dc}") for dc in range(ND)]
                depT = [ppool.tile([P, S], F16, name=f"depT{dc}",
                                   tag=f"depT{dc}") for dc in range(ND)]
                for h in range(2):
                    cols = slice(h * (S // 2), (h + 1) * (S // 2))
                    for dc in range(ND):
                        nc.scalar.dma_start(
                            headT[dc][:, cols],
                            headT_d[dc * P:(dc + 1) * P, cols])
                for h in range(2):
                    cols = slice(h * (S // 2), (h + 1) * (S // 2))
                    for dc in range(ND):
                        nc.scalar.dma_start(
                            depT[dc][:, cols],
                            depT_d[dc * P:(dc + 1) * P, cols])

                # ---- PE warm-up fillers on a memset tile (no DMA deps;
                #      dedicated PSUM bank so nothing waits on eviction) ----
                warm = cpool.tile([P, OC], F16, name="warm", tag="warm")
                nc.vector.memset(warm[:], 0.0)
                pw = psw.tile([P, OC], F32, name="psw", tag="psw")
                for _ in range(NWARM):
                    nc.tensor.matmul(pw[:], warm[:, 0:P], warm[:],
                                     start=True, stop=True)

                # ---- ATf[e, i] = U^T @ headT + wd (bias in eviction) ----
                atf = [ppool.tile([P, S], F16, name=f"atf{eb}", tag=f"atf{eb}")
                       for eb in range(ND)]

                def atf_chunk(ic):
                    for eb in range(ND):
                        pa = ps.tile([P, OC], F32, name="ps", tag="ps")
                        for dc in range(ND):
                            nc.tensor.matmul(
                                pa[:],
                                u2[:, dc * D + eb * P: dc * D + (eb + 1) * P],
                                headT[dc][:, ic * OC:(ic + 1) * OC],
                                start=(dc == 0), stop=(dc == ND - 1),
                            )
                        dst = atf[eb][:, ic * OC:(ic + 1) * OC]
                        if eb == 0:
                            nc.vector.tensor_scalar_add(
                                dst, pa[:], wd2[:, eb:eb + 1])
                        else:
                            nc.scalar.activation(
                                dst, pa[:], Ident, bias=wd2[:, eb:eb + 1])

                # out row-block: c_outer=True finishes chunk c before
                # touching chunk c+1 (2 LDW/chunk) -- used for the first
                # rows so they can start on the first dep half; eb-outer
                # (2 LDW/row) for the steady state.
                def out_row(ib, c_outer=False):
                    ot = outbuf.tile([P, S], F16, name="ot", tag="ot")
                    pos = [ps.tile([P, OC], F32, name="ps", tag="ps")
                           for _ in range(NC)]
                    if c_outer:
                        order = [(eb, c) for c in range(NC) for eb in range(ND)]
                    else:
                        order = [(eb, c) for eb in range(ND) for c in range(NC)]
                    for eb, c in order:
                        nc.tensor.matmul(
                            pos[c][:],
                            atf[eb][:, ib * P:(ib + 1) * P],
                            depT[eb][:, c * OC:(c + 1) * OC],
                            start=(eb == 0), stop=(eb == ND - 1),
                        )
                    for c in range(NC):
                        dst = ot[:, c * OC:(c + 1) * OC]
                        if c % 2 == 0:
                            nc.vector.tensor_scalar_add(
                                dst, pos[c][:], hs2[:, ib:ib + 1])
                        else:
                            nc.scalar.activation(
                                dst, pos[c][:], Ident, bias=hs2[:, ib:ib + 1])
                    nc.sync.dma_start(out_d[ib * P:(ib + 1) * P, :], ot[:])

                # ic0/ic1 gate rows 0-7; rows 0-3 are emitted right after
                # them so the out phase starts as soon as dep lands, with
                # ic2/ic3 queued behind as PE filler.
                atf_chunk(0)
                atf_chunk(1)
                for ib in range(4):
                    out_row(ib, c_outer=(ib < 2))
                atf_chunk(2)
                atf_chunk(3)
                for ib in range(4, NI):
                    out_row(ib)

            if reps > 1:
                with tc.For_i(0, reps, 1):
                    body()
            else:
                body()

    nc.finalize()
    return nc


_NC_CACHE = {}


def _get_nc(reps=1):
    if reps not in _NC_CACHE:
        _NC_CACHE[reps] = build_nc(reps)
    return _NC_CACHE[reps]


def make_in_maps(head, dep, edge_U, edge_W, edge_b):
    head = np.asarray(head, np.float32)
    dep = np.asarray(dep, np.float32)
    headT = np.ascontiguousarray(
        head.astype(np.float16).transpose(0, 2, 1))        # [B, D, S]
    depT = np.ascontiguousarray(
        dep.astype(np.float16).transpose(0, 2, 1))
    u = np.asarray(edge_U, np.float32).astype(np.float16)
    u2 = np.ascontiguousarray(
        np.concatenate([u[dc * P:(dc + 1) * P, :] for dc in range(ND)],
                       axis=1))                             # [128, 512]
    w = np.asarray(edge_W, np.float32).reshape(-1)
    wh, wd = w[:D], w[D:]
    wd2 = np.ascontiguousarray(wd.reshape(ND, P).T.astype(np.float32))
    b0 = float(np.asarray(edge_b, np.float32).reshape(-1)[0])
    hs = head @ wh + b0                                     # [B, S] f32
    hs2 = np.ascontiguousarray(
        hs.reshape(B, NI, P).transpose(0, 2, 1))            # [B, 128, 16]
    return [
        {"headT": headT[b], "depT": depT[b], "u2": u2, "wd2": wd2,
         "hs2": hs2[b]}
        for b in range(B)
    ]


def kernel(head, dep, edge_U, edge_W, edge_b):
    nc = _get_nc()
    in_maps = make_in_maps(head, dep, edge_U, edge_W, edge_b)
    last_err = None
    for _ in range(3):  # transient device errors happen on this shared env
        try:
            res = run_bass_kernel_spmd(nc, in_maps, core_ids=list(range(B)))
            break
        except Exception as e:  # noqa: BLE001
            last_err = e
    else:
        raise last_err
    return np.stack(
        [res.results[b]["out"].astype(np.float32) for b in range(B)], axis=0)
